# revision 9
# baseline (speedup 1.0000x reference)
"""Bass/Trainium2 kernel for BayesianDropoutLayer:
    out = X @ (mask[:, None] * M) + m
  X [8192, 2048] f32, M [2048, 2048] f32, m [2048] f32, mask [2048] i32.

Strategy (v2): data-parallel over batch across 8 NeuronCores, with
host-side contraction pruning and a DMA-paced PE schedule.

  - mask zeroes ~10% of M's rows; those k-rows contribute exactly 0, so the
    host gathers only the surviving rows (padded to a multiple of 128) of
    both M and X^T. K drops 2048 -> ~1920 (15 k-tiles), cutting PE work and
    load traffic by 1/16 and removing all on-device mask handling.
  - orientation: stationary = M subtile [128k, 128u], moving = X^T chunk
    [128k, 512b]; out tiles are [128u, 512b] (units on partitions). Bias is
    then a per-partition scalar added during PSUM eviction (no PE broadcast).
  - 4 unit-phases of 512 units; within phases 0-2 the 8 PSUM chains are
    accumulated kt-OUTER (interleaved across banks) so the PE consumes each
    (mw_kt, xt_kt) pair as it arrives from HBM; the single sync-queue load
    stream is ordered [mw0_0, xt_0, mw0_1, xt_1, ...] to match. The last
    phase runs kt-inner so chains finish staggered and the final evictions
    / stores hide behind remaining matmuls.
  - evictions alternate Vector/Scalar engines (PSUM -> SBUF + bias add) so
    a phase's 8 evictions finish before the next phase needs the banks;
    stores go out on the (idle) GpSimd engine's HWDGE queue per u-tile.
  - a few warmup matmuls on a memset tile run during the DMA head so the
    PE's HAM clock-gate is released before the real stream starts.
"""

import sys

if "/opt/trn_rl_repo" not in sys.path:
    sys.path.insert(0, "/opt/trn_rl_repo")

import numpy as np

import concourse.bass as bass  # noqa: F401  (registers sys modules)
import concourse.mybir as mybir
import concourse.tile as tile
from concourse import bacc
from concourse.bass_utils import run_bass_kernel_spmd

P = 128
BATCH = 8192
N_IN = 2048
UNITS = 2048
N_CORES = 8
B_SHARD = BATCH // N_CORES          # 1024 rows per core
NPAN = 4                            # unit phases
UP = UNITS // NPAN                  # 512 units per phase
NB = 512                            # moving-dim chunk (PSUM bank = 512 f32)
NUT = UNITS // P                    # 16 u-tiles

F32 = mybir.dt.float32
F32R = mybir.dt.float32r

_CACHED = {}


def _build_nc(n_kt):
    if n_kt in _CACHED:
        return _CACHED[n_kt]

    k_pad = n_kt * P
    nc = bacc.Bacc("TRN2", target_bir_lowering=False, debug=False)

    xt = nc.dram_tensor("xt", [k_pad, B_SHARD], F32R, kind="ExternalInput")
    mw = nc.dram_tensor("mw", [k_pad, UNITS], F32R, kind="ExternalInput")
    biasd = nc.dram_tensor("biasd", [P, NUT], F32, kind="ExternalInput")
    # out[ut, bc, p, n] = result for unit ut*128+p, batch-row bc*512+n: each
    # (u-tile, batch-chunk) store is one fully contiguous 256 KiB block, and
    # the final store (on the kernel's tail) is small.
    out = nc.dram_tensor("out", [NUT, 2, P, NB], F32, kind="ExternalOutput")

    xt3 = xt.rearrange("(kt p) b -> p kt b", p=P)
    mw3 = mw.rearrange("(kt p) n -> p kt n", p=P)

    # 4-kt batches for phases 1..3 (fewer sync-engine DMA triggers)
    groups = []
    g0 = 0
    while g0 < n_kt:
        gs = min(4, n_kt - g0)
        groups.append((g0, gs))
        g0 += gs

    # phase-0 load batching ramps from single k-tiles (first data lands
    # fast, PE starts early) to 4-kt batches (the sync engine pays ~1us
    # per DMA trigger; fine-grained singles throttled the stream to
    # ~330 GB/s vs the ~400 GB/s a single queue sustains)
    ramp = []
    r0 = 0
    for sz in (1, 1, 1, 2, 2, 4, 4, 4, 4, 4, 4):
        if r0 >= n_kt:
            break
        sz = min(sz, n_kt - r0)
        ramp.append((r0, sz))
        r0 += sz

    with tile.TileContext(nc) as tc:
        with (
            tc.tile_pool(name="xtp", bufs=1) as xtp,
            tc.tile_pool(name="mwp", bufs=1) as mwp,
            tc.tile_pool(name="mwgp", bufs=1) as mwgp,
            tc.tile_pool(name="misc", bufs=1) as misc,
            tc.tile_pool(name="outp", bufs=1) as outp,
            tc.tile_pool(name="psum", bufs=8, space="PSUM") as psump,
        ):
            bias_sb = misc.tile([P, NUT], F32)
            nc.scalar.dma_start(bias_sb[:], biasd[:, :])
            wt = misc.tile([P, NB], F32R)
            # walrus rejects memset on f32r; write zeros through a u32 view
            nc.vector.memset(wt[:].bitcast(mybir.dt.uint32), 0)

            # PE warmup during the DMA head: ~3.4us of activity releases the
            # HAM clock gate right as the first real matmuls start.
            wps = psump.tile([P, NB], F32, tag="ps", name="wps")
            for _ in range(8):
                nc.tensor.matmul(wps[:], wt[:, 0:P], wt[:, :], start=True, stop=True)

            # ---- load stream (single sync queue, arrival-paced) ----
            # phase-0 interleave: the PE's kt-th step needs exactly
            # (mw0_kt, xt_kt); later-phase mw batches follow behind.
            mw_tiles = {pn: [None] * n_kt for pn in range(NPAN)}
            xts = []
            for (r0, rsz) in ramp:
                m0 = mwp.tile([P, rsz, UP], F32R, name=f"mw0_{r0}")
                nc.sync.dma_start(m0[:], mw3[:, r0 : r0 + rsz, 0:UP])
                for j in range(rsz):
                    mw_tiles[0][r0 + j] = m0[:, j, :]
                x = xtp.tile([P, rsz, B_SHARD], F32R, name=f"xt_{r0}")
                nc.sync.dma_start(x[:], xt3[:, r0 : r0 + rsz, :])
                for j in range(rsz):
                    xts.append(x[:, j, :])
            for pn in range(1, NPAN):
                n0 = pn * UP
                for (gs0, gsz) in groups:
                    t = mwgp.tile(
                        [P, gsz, UP], F32R, tag="mwg", bufs=8,
                        name=f"mwg{pn}_{gs0}",
                    )
                    nc.sync.dma_start(t[:], mw3[:, gs0 : gs0 + gsz, n0 : n0 + UP])
                    for j in range(gsz):
                        mw_tiles[pn][gs0 + j] = t[:, j, :]

            def evict_store(pn, ul, ps_pair):
                ut = pn * 4 + ul
                # split PSUM->SBUF evictions across two engines so all 8 of a
                # phase finish before the next phase reuses the banks; store
                # each 256 KiB chunk as soon as its eviction lands
                for bc in range(2):
                    ob = outp.tile(
                        [P, NB], F32, tag="ob", bufs=8, name=f"ob{ut}_{bc}"
                    )
                    if ul % 2 == 0:
                        nc.vector.tensor_scalar_add(
                            ob[:], ps_pair[bc][:], bias_sb[:, ut : ut + 1]
                        )
                    else:
                        nc.scalar.add(
                            ob[:], ps_pair[bc][:], bias_sb[:, ut : ut + 1]
                        )
                    nc.gpsimd.dma_start(out[ut, bc, :, :], ob[:])

            for pn in range(NPAN):
                mwt = mw_tiles[pn]
                ps = [
                    psump.tile([P, NB], F32, tag="ps", name=f"ps{pn}_{i}")
                    for i in range(8)
                ]
                if pn < NPAN - 1:
                    # kt-outer: 8 interleaved full-K chains consume each
                    # k-tile as it lands
                    for kt in range(n_kt):
                        st = kt == 0
                        sp = kt == n_kt - 1
                        for ul in range(4):
                            lhsT = mwt[kt][:, ul * P : (ul + 1) * P]
                            nc.tensor.matmul(
                                ps[2 * ul][:], lhsT, xts[kt][:, 0:NB],
                                start=st, stop=sp,
                            )
                            nc.tensor.matmul(
                                ps[2 * ul + 1][:], lhsT, xts[kt][:, NB : 2 * NB],
                                start=st, stop=sp,
                            )
                    for ul in range(4):
                        evict_store(pn, ul, (ps[2 * ul], ps[2 * ul + 1]))
                else:
                    # final phase kt-inner: chains finish staggered so the
                    # last evictions/stores hide behind remaining matmuls
                    for ul in range(4):
                        for kt in range(n_kt):
                            st = kt == 0
                            sp = kt == n_kt - 1
                            lhsT = mwt[kt][:, ul * P : (ul + 1) * P]
                            nc.tensor.matmul(
                                ps[2 * ul][:], lhsT, xts[kt][:, 0:NB],
                                start=st, stop=sp,
                            )
                            nc.tensor.matmul(
                                ps[2 * ul + 1][:], lhsT, xts[kt][:, NB : 2 * NB],
                                start=st, stop=sp,
                            )
                        evict_store(pn, ul, (ps[2 * ul], ps[2 * ul + 1]))

    nc.compile()
    _CACHED[n_kt] = nc
    return nc


def _prep(X, M, m, mask):
    """Host-side pruning/layout. Returns (n_kt, idx, mw, bias2d)."""
    mask = np.asarray(mask, dtype=np.int32).reshape(N_IN)
    keep = np.flatnonzero(mask != 0)
    n_kt = max(1, -(-len(keep) // P))
    k_pad = n_kt * P
    if len(keep) < k_pad:
        pad = np.flatnonzero(mask == 0)[: k_pad - len(keep)]
        idx = np.concatenate([keep, pad])
    else:
        idx = keep
    mw = np.ascontiguousarray(np.asarray(M, dtype=np.float32)[idx])
    if len(keep) < k_pad:
        mw[len(keep):] = 0.0
    bias2d = np.ascontiguousarray(
        np.asarray(m, dtype=np.float32).reshape(NUT, P).T
    )
    return n_kt, idx, mw, bias2d


def run_sharded(X, M, m, mask, trace=False, trace_cores=None):
    """Returns (full_output, BassKernelResults)."""
    n_kt, idx, mw, bias2d = _prep(X, M, m, mask)
    nc = _build_nc(n_kt)
    X = np.asarray(X, dtype=np.float32)
    in_maps = []
    for c in range(N_CORES):
        xs = X[c * B_SHARD : (c + 1) * B_SHARD]
        xtc = np.ascontiguousarray(xs.T[idx])  # [k_pad, B_SHARD]
        in_maps.append({"xt": xtc, "mw": mw, "biasd": bias2d})
    res = run_bass_kernel_spmd(
        nc,
        in_maps,
        list(range(N_CORES)),
        trace=trace,
        trace_cores=trace_cores,
    )
    shards = [
        np.transpose(r["out"], (1, 3, 0, 2)).reshape(B_SHARD, UNITS)
        for r in res.results
    ]
    out = np.ascontiguousarray(np.concatenate(shards, axis=0))
    return out, res


def kernel(X, M, m, mask):
    out, _ = run_sharded(X, M, m, mask)
    return out
